# revision 6
# baseline (speedup 1.0000x reference)
"""Trainium2 Bass kernel for 1D cubic B-spline evaluation with linear
extrapolation (nn_BSpline1D).

Math: the reference spline (16 clamped-uniform basis functions, degree 3 on
[0,1]) is a piecewise cubic over 13 uniform spans.  With w = clamp(13*x, 0, 13)
and two anchor regions (w in [0,7) and [7,14)), the spline is evaluated
exactly as

    y = C_r(v) + sum_j E_{r,j} * relu(v - j)^3        v = w - 7*r, j = 1..6

where C_r is the cubic of the region's first span (continued) and E_{r,j} are
the third-derivative jumps at the region's interior knots (anchored truncated
power form; all intermediates O(1), fp32-stable).  Region selection is a
single is_ge mask; per-element region tables are affine in the mask.  Linear
extrapolation is folded in branchlessly:

    y += slope_lo * min(x, 0) + slope_hi * (max(x, 1) - 1)

Sharding: embarrassingly data-parallel; x split evenly across 8 NeuronCores.
"""
import sys

sys.path.insert(0, "/opt/trn_rl_repo")

import numpy as np

N_BASIS = 16
DEGREE = 3
EPS_DENOM = 1e-12
NSEG = N_BASIS - DEGREE          # 13 spans
W = 7                            # region width (knots), 2 regions
R = 2

N_CORES = 8
TOTAL = 8388608
PTS = TOTAL // N_CORES           # 1048576 per core
P = 128
F = 1024
NT = PTS // (P * F)              # tiles per core


# ---------------------------------------------------------------- host math

def _bspline_basis(x, knots):
    """fp64 replica of the reference Cox-de Boor basis."""
    x = np.asarray(x, np.float64)
    knots = np.asarray(knots, np.float64)
    xk = x[:, None]
    left_k = knots[:N_BASIS]
    right_k = knots[1:N_BASIS + 1]
    B = ((xk >= left_k) & (xk < right_k)).astype(np.float64)
    last = ((x >= knots[N_BASIS - 1]) & (x <= knots[N_BASIS])).astype(np.float64)
    B[:, -1] = last
    for p in range(1, DEGREE + 1):
        d1 = knots[p:p + N_BASIS] - knots[:N_BASIS]
        d2 = knots[p + 1:p + 1 + N_BASIS] - knots[1:1 + N_BASIS]
        inv1 = np.where(np.abs(d1) > EPS_DENOM, 1.0 / np.where(np.abs(d1) > EPS_DENOM, d1, 1.0), 0.0)
        inv2 = np.where(np.abs(d2) > EPS_DENOM, 1.0 / np.where(np.abs(d2) > EPS_DENOM, d2, 1.0), 0.0)
        B_shift = np.pad(B[:, 1:], ((0, 0), (0, 1)))
        B = (xk - knots[:N_BASIS]) * inv1 * B + (knots[p + 1:p + 1 + N_BASIS] - xk) * inv2 * B_shift
    return B


def _build_tables(coeffs, knots):
    coeffs = np.asarray(coeffs, np.float64)
    knots = np.asarray(knots, np.float64)
    h = 1.0 / NSEG

    # per-span local cubic g[s, c] in u = 13*x - s via exact fp64 fit
    us = np.array([0.125, 0.375, 0.625, 0.875])
    Vinv = np.linalg.inv(np.vander(us, 4, increasing=True))
    g = np.zeros((NSEG, 4))
    for s in range(NSEG):
        xs = (s + us) * h
        g[s] = Vinv @ (_bspline_basis(xs, knots) @ coeffs)

    e = np.zeros(NSEG)
    e[1:] = g[1:, 3] - g[:-1, 3]          # u^3 coefficient jump at knot t

    G = np.zeros((R, 4))
    K = np.zeros((R, W - 1))
    for r in range(R):
        G[r] = g[min(r * W, NSEG - 1)]
        for j in range(1, W):
            t = r * W + j
            if t < NSEG:
                K[r, j - 1] = np.cbrt(e[t])

    def ev(t):
        return float((_bspline_basis(np.array([t]), knots) @ coeffs)[0])

    slope_lo = (ev(0.001) - ev(0.0)) / (0.001 + EPS_DENOM)
    slope_hi = (ev(1.0) - ev(0.999)) / (0.001 + EPS_DENOM)
    return G, K, slope_lo, slope_hi


# ---------------------------------------------------------------- device kernel

def _build_nc(G, K, slope_lo, slope_hi, nrep=1):
    import concourse.bacc as bacc
    import concourse.mybir as mybir
    from concourse import tile

    dt = mybir.dt.float32
    op = mybir.AluOpType
    act = mybir.ActivationFunctionType

    f32 = lambda v: float(np.float32(v))

    nc = bacc.Bacc("TRN2", target_bir_lowering=False, debug=False, num_devices=N_CORES)
    x_ext = nc.dram_tensor("x", [PTS], dt, kind="ExternalInput")
    y_ext = nc.dram_tensor("y", [PTS], dt, kind="ExternalOutput")
    xv = x_ext.ap().rearrange("(n p f) -> n p f", p=P, f=F)
    yv = y_ext.ap().rearrange("(n p f) -> n p f", p=P, f=F)

    # stream tables: value = base + m * delta  (m = region mask in {0,1})
    bases = [f32(G[0, c]) for c in range(4)] + [f32(K[0, j]) for j in range(W - 1)]
    deltas = [f32(G[1, c] - G[0, c]) for c in range(4)] + \
             [f32(K[1, j] - K[0, j]) for j in range(W - 1)]

    with tile.TileContext(nc) as tc:
        with (
            tc.tile_pool(name="cp", bufs=1) as cpool,
            tc.tile_pool(name="io", bufs=3) as iop,
            tc.tile_pool(name="mid", bufs=2) as midp,
            tc.tile_pool(name="tmp", bufs=3) as tmpp,
        ):
            # bias constants: 10 stream bases + relu offsets -1..-6
            ncol = len(bases) + (W - 1)
            consts = cpool.tile([P, ncol], dt)
            for i, b in enumerate(bases):
                nc.gpsimd.memset(consts[:, i:i + 1], b)
            for j in range(1, W):
                nc.gpsimd.memset(consts[:, len(bases) + j - 1:len(bases) + j], float(-j))

            def base_ap(i):
                return consts[:, i:i + 1]

            def relu_bias(j):
                return consts[:, len(bases) + j - 1:len(bases) + j]

            for it in [i for _ in range(nrep) for i in range(NT)]:
                xt = iop.tile([P, F], dt, tag="x")
                nc.sync.dma_start(xt[:], xv[it])

                w1 = midp.tile([P, F], dt, tag="w1")
                nc.vector.tensor_scalar(w1[:], xt[:], 13.0, 0.0, op.mult, op.max)
                m = midp.tile([P, F], dt, tag="m")
                nc.vector.tensor_scalar(m[:], w1[:], float(W), None, op.is_ge)
                # wc = w1 min 13 (in place), then v = m*(-W) + wc
                nc.vector.tensor_scalar(w1[:], w1[:], 13.0, None, op.min)
                v = midp.tile([P, F], dt, tag="v")
                nc.vector.scalar_tensor_tensor(v[:], m[:], float(-W), w1[:], op.mult, op.add)

                # gather region tables (ACT affine of the mask)
                cs = []
                for c in range(4):
                    ct = midp.tile([P, F], dt, tag=f"c{c}")
                    nc.scalar.activation(ct[:], m[:], act.Identity, bias=base_ap(c), scale=deltas[c])
                    cs.append(ct)

                # Horner on the base cubic; h doubles as the y accumulator
                h = iop.tile([P, F], dt, tag="y")
                nc.vector.tensor_tensor(h[:], cs[3][:], v[:], op.mult)
                nc.vector.tensor_tensor(h[:], h[:], cs[2][:], op.add)
                nc.vector.tensor_tensor(h[:], h[:], v[:], op.mult)
                nc.vector.tensor_tensor(h[:], h[:], cs[1][:], op.add)
                nc.vector.tensor_tensor(h[:], h[:], v[:], op.mult)
                nc.vector.tensor_tensor(h[:], h[:], cs[0][:], op.add)

                # knot corrections: y += (kappa_j * relu(v-j))^3
                for j in range(1, W):
                    kj = tmpp.tile([P, F], dt, tag="kap")
                    nc.scalar.activation(kj[:], m[:], act.Identity,
                                         bias=base_ap(4 + j - 1), scale=deltas[4 + j - 1])
                    rj = tmpp.tile([P, F], dt, tag="relu")
                    nc.scalar.activation(rj[:], v[:], act.Relu, bias=relu_bias(j), scale=1.0)
                    tj = tmpp.tile([P, F], dt, tag="t")
                    nc.vector.tensor_tensor(tj[:], rj[:], kj[:], op.mult)
                    sj = tmpp.tile([P, F], dt, tag="sq")
                    nc.scalar.activation(sj[:], tj[:], act.Square)
                    cu = tmpp.tile([P, F], dt, tag="cu")
                    nc.vector.tensor_tensor(cu[:], sj[:], tj[:], op.mult)
                    nc.vector.tensor_tensor(h[:], h[:], cu[:], op.add)

                # branchless linear extrapolation
                elo = midp.tile([P, F], dt, tag="elo")
                nc.vector.tensor_scalar(elo[:], xt[:], 0.0, None, op.min)
                nc.vector.scalar_tensor_tensor(h[:], elo[:], f32(slope_lo), h[:], op.mult, op.add)
                ehi = midp.tile([P, F], dt, tag="ehi")
                nc.vector.tensor_scalar(ehi[:], xt[:], 1.0, 1.0, op.max, op.subtract)
                nc.vector.scalar_tensor_tensor(h[:], ehi[:], f32(slope_hi), h[:], op.mult, op.add)

                nc.sync.dma_start(yv[it], h[:])

    nc.compile()
    return nc


def _run(x, coeffs, knots, trace=False, nrep=1, **kw):
    from concourse.bass_utils import run_bass_kernel_spmd

    x = np.ascontiguousarray(np.asarray(x, np.float32).reshape(-1))
    assert x.size == TOTAL, x.size
    G, K, slope_lo, slope_hi = _build_tables(coeffs, knots)
    nc = _build_nc(G, K, slope_lo, slope_hi, nrep=nrep)

    shards = x.reshape(N_CORES, PTS)
    in_maps = [{"x": shards[i]} for i in range(N_CORES)]
    res = run_bass_kernel_spmd(nc, in_maps, core_ids=list(range(N_CORES)),
                               trace=trace, **kw)
    y = np.concatenate([np.asarray(res.results[i]["y"], np.float32).reshape(-1)
                        for i in range(N_CORES)])
    return y.reshape(-1, 1), res


def kernel(x, coeffs, knots):
    return _run(x, coeffs, knots)[0]


# revision 7
# speedup vs baseline: 187.7440x; 187.7440x over previous
"""Trainium2 Bass kernel for 1D cubic B-spline evaluation with linear
extrapolation (nn_BSpline1D).

Math: the reference spline (16 clamped-uniform basis, degree 3 on [0,1]) is a
piecewise cubic over 13 uniform spans.  With w = clamp(13*x, 0, 13) and
v = w - 6 (anchored at span 6), the spline is exactly

    y = C(v) + sum_{t=7..12} e_t relu(v - (t-6))^3
             + sum_{t=1..6}  e_t relu((t-6) - v)^3

where C is span 6's cubic continued in both directions and e_t are the
third-derivative jumps at the interior knots (truncated power form; the
downward continuation flips (w-t)^3 = -(t-w)^3, so both sides enter with
sign(e_t)).  Up/down arms have disjoint supports, so a same-sign up/down pair
(k1*relu(v-a) + k2*relu(b-v))^3 equals the sum of the two cubes; each pair's
two-arm piecewise-linear "flat-bottom V" is built in two ScalarE ops
(Prelu then scaled Relu), cubed via Square + one multiply, and accumulated
with a tensor add/subtract.  Linear extrapolation is folded in branchlessly
the same way (degree-1 arms at x=0 and x=1).

Sharding: embarrassingly data-parallel; x split evenly across 8 NeuronCores.
"""
import sys

sys.path.insert(0, "/opt/trn_rl_repo")

import numpy as np

N_BASIS = 16
DEGREE = 3
EPS_DENOM = 1e-12
NSEG = N_BASIS - DEGREE          # 13 spans

N_CORES = 8
TOTAL = 8388608
PTS = TOTAL // N_CORES           # 1048576 per core
P = 128
F = 2048
NT = PTS // (P * F)              # tiles per core


# ---------------------------------------------------------------- host math

def _bspline_basis(x, knots):
    """fp64 replica of the reference Cox-de Boor basis."""
    x = np.asarray(x, np.float64)
    knots = np.asarray(knots, np.float64)
    xk = x[:, None]
    left_k = knots[:N_BASIS]
    right_k = knots[1:N_BASIS + 1]
    B = ((xk >= left_k) & (xk < right_k)).astype(np.float64)
    last = ((x >= knots[N_BASIS - 1]) & (x <= knots[N_BASIS])).astype(np.float64)
    B[:, -1] = last
    for p in range(1, DEGREE + 1):
        d1 = knots[p:p + N_BASIS] - knots[:N_BASIS]
        d2 = knots[p + 1:p + 1 + N_BASIS] - knots[1:1 + N_BASIS]
        inv1 = np.where(np.abs(d1) > EPS_DENOM, 1.0 / np.where(np.abs(d1) > EPS_DENOM, d1, 1.0), 0.0)
        inv2 = np.where(np.abs(d2) > EPS_DENOM, 1.0 / np.where(np.abs(d2) > EPS_DENOM, d2, 1.0), 0.0)
        B_shift = np.pad(B[:, 1:], ((0, 0), (0, 1)))
        B = (xk - knots[:N_BASIS]) * inv1 * B + (knots[p + 1:p + 1 + N_BASIS] - xk) * inv2 * B_shift
    return B


def _plan(coeffs, knots):
    """Build the evaluation plan: base cubic, paired/single cubic arms,
    extrapolation arms."""
    coeffs = np.asarray(coeffs, np.float64)
    knots = np.asarray(knots, np.float64)
    h = 1.0 / NSEG

    us = np.array([0.125, 0.375, 0.625, 0.875])
    Vinv = np.linalg.inv(np.vander(us, 4, increasing=True))
    g = np.zeros((NSEG, 4))
    for s in range(NSEG):
        xs = (s + us) * h
        g[s] = Vinv @ (_bspline_basis(xs, knots) @ coeffs)
    e = np.zeros(NSEG)
    e[1:] = g[1:, 3] - g[:-1, 3]
    C = [float(c) for c in g[6]]

    # cubic arms in v = w - 6: ("up"/"dn", v_t, kappa, sigma)
    arms = []
    for t in range(7, 13):
        if e[t] != 0.0:
            arms.append(("up", float(t - 6), float(np.cbrt(abs(e[t]))), 1.0 if e[t] > 0 else -1.0))
    for t in range(1, 7):
        if e[t] != 0.0:
            arms.append(("dn", float(t - 6), float(np.cbrt(abs(e[t]))), 1.0 if e[t] > 0 else -1.0))

    ups = sorted([a for a in arms if a[0] == "up"], key=lambda a: a[1])
    dns = sorted([a for a in arms if a[0] == "dn"], key=lambda a: -a[1])
    pairs, singles = [], []
    used = [False] * len(dns)
    for u in ups:
        for i, d in enumerate(dns):
            if not used[i] and d[3] == u[3]:
                used[i] = True
                pairs.append((u, d))
                break
        else:
            singles.append(u)
    singles += [d for i, d in enumerate(dns) if not used[i]]

    def ev(t):
        return float((_bspline_basis(np.array([t]), knots) @ coeffs)[0])

    slope_lo = (ev(0.001) - ev(0.0)) / (0.001 + EPS_DENOM)
    slope_hi = (ev(1.0) - ev(0.999)) / (0.001 + EPS_DENOM)
    return C, pairs, singles, slope_lo, slope_hi


# ---------------------------------------------------------------- device kernel

def _build_nc(plan, nrep=1):
    import concourse.bacc as bacc
    import concourse.mybir as mybir
    from concourse import tile

    dt = mybir.dt.float32
    op = mybir.AluOpType
    act = mybir.ActivationFunctionType

    C, pairs, singles, slope_lo, slope_hi = plan
    f32 = lambda v: float(np.float32(v))

    # --- precompute all ACT parameters (host, fp64 -> fp32) ---
    # cubic arm groups: list of dicts describing the ACT chain per group
    groups = []     # each: {kind: pair|single, sigma, params...}
    for (u, d) in pairs:
        _, a_vt, k1, sig = u
        _, b_vt, k2, _ = d
        alpha = -(k2 / k1)
        m = (a_vt * k1 + b_vt * k2) / (k1 + k2)
        groups.append(dict(kind="pair", sigma=sig, m=f32(m), alpha=f32(alpha),
                           k1=f32(k1), rbias=f32(-k1 * (a_vt - m))))
    for (side, vt, kap, sig) in singles:
        scale = kap if side == "up" else -kap
        rbias = -kap * vt if side == "up" else kap * vt
        groups.append(dict(kind="single", sigma=sig, scale=f32(scale), rbias=f32(rbias)))

    # extrapolation arms (degree 1): coeff_lo = -slope_lo on relu(-x),
    # coeff_hi = slope_hi on relu(x-1)
    c_lo, c_hi = -slope_lo, slope_hi
    ext = []
    if c_lo != 0.0 and c_hi != 0.0 and (c_lo > 0) == (c_hi > 0):
        klo, khi, sig = abs(c_lo), abs(c_hi), 1.0 if c_lo > 0 else -1.0
        m = khi / (khi + klo)          # (a*khi + b*klo)/(khi+klo), a=1, b=0
        ext.append(dict(kind="pair", sigma=sig, m=f32(m), alpha=f32(-klo / khi),
                        k1=f32(khi), rbias=f32(-khi * (1.0 - m))))
    else:
        if c_lo != 0.0:
            ext.append(dict(kind="single", sigma=1.0 if c_lo > 0 else -1.0,
                            scale=f32(-abs(c_lo)), rbias=0.0))
        if c_hi != 0.0:
            ext.append(dict(kind="single", sigma=1.0 if c_hi > 0 else -1.0,
                            scale=f32(abs(c_hi)), rbias=f32(-abs(c_hi))))

    # collect non-trivial bias constants -> consts tile columns
    bias_vals = []
    def bias_col(val):
        val = f32(val)
        if val not in bias_vals:
            bias_vals.append(val)
        return bias_vals.index(val)
    for grp in groups:
        if grp["kind"] == "pair":
            grp["mcol"] = bias_col(-grp["m"])
        grp["rcol"] = bias_col(grp["rbias"])
    for grp in ext:
        if grp["kind"] == "pair":
            grp["mcol"] = bias_col(-grp["m"])
        grp["rcol"] = bias_col(grp["rbias"])

    nc = bacc.Bacc("TRN2", target_bir_lowering=False, debug=False, num_devices=N_CORES)
    x_ext = nc.dram_tensor("x", [PTS], dt, kind="ExternalInput")
    y_ext = nc.dram_tensor("y", [PTS], dt, kind="ExternalOutput")
    xv = x_ext.ap().rearrange("(n p f) -> n p f", p=P, f=F)
    yv = y_ext.ap().rearrange("(n p f) -> n p f", p=P, f=F)

    with tile.TileContext(nc) as tc:
        with (
            tc.tile_pool(name="cp", bufs=1) as cpool,
            tc.tile_pool(name="io", bufs=3) as iop,
            tc.tile_pool(name="mid", bufs=2) as midp,
            tc.tile_pool(name="tmp", bufs=3) as tmpp,
        ):
            consts = cpool.tile([P, max(len(bias_vals), 1)], dt)
            for i, b in enumerate(bias_vals):
                nc.gpsimd.memset(consts[:, i:i + 1], b)

            def bias_ap(col):
                return consts[:, col:col + 1]

            for it in [i for _ in range(nrep) for i in range(NT)]:
                xt = iop.tile([P, F], dt, tag="x")
                nc.sync.dma_start(xt[:], xv[it])

                # w1 = relu(13 x) on ACT; v = (w1 min 13) - 6 on DVE
                w1 = midp.tile([P, F], dt, tag="w1")
                nc.scalar.activation(w1[:], xt[:], act.Relu, bias=0.0, scale=13.0)
                v = midp.tile([P, F], dt, tag="v")
                nc.vector.tensor_scalar(v[:], w1[:], 13.0, 6.0, op.min, op.subtract)

                # base cubic Horner in h (in place):
                # h = (g3 v + g2); h = h*v; h = (h + g1)*v;  (g0 folded below)
                h = midp.tile([P, F], dt, tag="h")
                nc.vector.tensor_scalar(h[:], v[:], f32(C[3]), f32(C[2]), op.mult, op.add)
                nc.vector.tensor_tensor(h[:], h[:], v[:], op.mult)
                nc.vector.scalar_tensor_tensor(h[:], h[:], f32(C[1]), v[:], op.add, op.mult)

                # cubic arm groups -> cubes
                cubes = []   # (cube_tile, sigma)
                for gi, grp in enumerate(groups):
                    if grp["kind"] == "pair":
                        p_t = tmpp.tile([P, F], dt, tag="p")
                        nc.scalar.activation(p_t[:], v[:], act.Prelu,
                                             bias=bias_ap(grp["mcol"]), scale=1.0,
                                             alpha=grp["alpha"])
                        r_t = tmpp.tile([P, F], dt, tag="r")
                        nc.scalar.activation(r_t[:], p_t[:], act.Relu,
                                             bias=bias_ap(grp["rcol"]), scale=grp["k1"])
                    else:
                        r_t = tmpp.tile([P, F], dt, tag="r")
                        nc.scalar.activation(r_t[:], v[:], act.Relu,
                                             bias=bias_ap(grp["rcol"]), scale=grp["scale"])
                    sq_t = tmpp.tile([P, F], dt, tag="sq")
                    nc.scalar.activation(sq_t[:], r_t[:], act.Square)
                    cu_t = tmpp.tile([P, F], dt, tag="cu")
                    nc.vector.tensor_tensor(cu_t[:], sq_t[:], r_t[:], op.mult)
                    cubes.append((cu_t, grp["sigma"]))

                # y = (h + g0) +/- cube_0, then accumulate the rest
                y = iop.tile([P, F], dt, tag="y")
                if cubes:
                    cu0, sig0 = cubes[0]
                    nc.vector.scalar_tensor_tensor(
                        y[:], h[:], f32(C[0]), cu0[:], op.add,
                        op.add if sig0 > 0 else op.subtract)
                    for cu_t, sig in cubes[1:]:
                        nc.vector.tensor_tensor(y[:], y[:], cu_t[:],
                                                op.add if sig > 0 else op.subtract)
                else:
                    nc.vector.tensor_scalar(y[:], h[:], f32(C[0]), None, op.add)

                # extrapolation arms (degree 1) on x
                for grp in ext:
                    if grp["kind"] == "pair":
                        p_t = tmpp.tile([P, F], dt, tag="p")
                        nc.scalar.activation(p_t[:], xt[:], act.Prelu,
                                             bias=bias_ap(grp["mcol"]), scale=1.0,
                                             alpha=grp["alpha"])
                        r_t = tmpp.tile([P, F], dt, tag="r")
                        nc.scalar.activation(r_t[:], p_t[:], act.Relu,
                                             bias=bias_ap(grp["rcol"]), scale=grp["k1"])
                    else:
                        r_t = tmpp.tile([P, F], dt, tag="r")
                        nc.scalar.activation(r_t[:], xt[:], act.Relu,
                                             bias=bias_ap(grp["rcol"]), scale=grp["scale"])
                    nc.vector.scalar_tensor_tensor(y[:], r_t[:], grp["sigma"], y[:],
                                                   op.mult, op.add)

                nc.sync.dma_start(yv[it], y[:])

    nc.compile()
    return nc


def _run(x, coeffs, knots, nrep=1, **kw):
    from concourse.bass_utils import run_bass_kernel_spmd

    x = np.ascontiguousarray(np.asarray(x, np.float32).reshape(-1))
    assert x.size == TOTAL, x.size
    plan = _plan(coeffs, knots)
    nc = _build_nc(plan, nrep=nrep)

    shards = x.reshape(N_CORES, PTS)
    in_maps = [{"x": shards[i]} for i in range(N_CORES)]
    res = run_bass_kernel_spmd(nc, in_maps, core_ids=list(range(N_CORES)), **kw)
    y = np.concatenate([np.asarray(res.results[i]["y"], np.float32).reshape(-1)
                        for i in range(N_CORES)])
    return y.reshape(-1, 1), res


def kernel(x, coeffs, knots):
    return _run(x, coeffs, knots)[0]


# revision 17
# speedup vs baseline: 197.8170x; 1.0537x over previous
"""Trainium2 Bass kernel for 1D cubic B-spline evaluation with linear
extrapolation (nn_BSpline1D).

Math: the reference spline (16 clamped-uniform basis, degree 3 on [0,1]) is a
piecewise cubic over 13 uniform spans.  With w = clamp(13*x, 0, 13) and
v = w - 6 (anchored at span 6), the spline is exactly

    y = C(v) + sum_{t=7..12} e_t relu(v - (t-6))^3
             + sum_{t=1..6}  e_t relu((t-6) - v)^3

where C is span 6's cubic continued in both directions and e_t are the
third-derivative jumps at the interior knots (truncated power form; the
downward continuation flips (w-t)^3 = -(t-w)^3, so both sides enter with
sign(e_t)).  Up/down arms have disjoint supports, so a same-sign up/down pair
(k1*relu(v-a) + k2*relu(b-v))^3 equals the sum of the two cubes; each pair's
two-arm piecewise-linear "flat-bottom V" is built in two ScalarE ops
(Prelu then scaled Relu), cubed via Square + one multiply, and accumulated
with a tensor add/subtract.  Linear extrapolation is folded in branchlessly
the same way (degree-1 arms at x=0 and x=1).

Sharding: embarrassingly data-parallel; x split evenly across 8 NeuronCores.
"""
import sys

sys.path.insert(0, "/opt/trn_rl_repo")

import numpy as np

N_BASIS = 16
DEGREE = 3
EPS_DENOM = 1e-12
NSEG = N_BASIS - DEGREE          # 13 spans

N_CORES = 8
TOTAL = 8388608
PTS = TOTAL // N_CORES           # 1048576 per core
P = 128
F = 2048
NT = PTS // (P * F)              # tiles per core


# ---------------------------------------------------------------- host math

def _bspline_basis(x, knots):
    """fp64 replica of the reference Cox-de Boor basis."""
    x = np.asarray(x, np.float64)
    knots = np.asarray(knots, np.float64)
    xk = x[:, None]
    left_k = knots[:N_BASIS]
    right_k = knots[1:N_BASIS + 1]
    B = ((xk >= left_k) & (xk < right_k)).astype(np.float64)
    last = ((x >= knots[N_BASIS - 1]) & (x <= knots[N_BASIS])).astype(np.float64)
    B[:, -1] = last
    for p in range(1, DEGREE + 1):
        d1 = knots[p:p + N_BASIS] - knots[:N_BASIS]
        d2 = knots[p + 1:p + 1 + N_BASIS] - knots[1:1 + N_BASIS]
        inv1 = np.where(np.abs(d1) > EPS_DENOM, 1.0 / np.where(np.abs(d1) > EPS_DENOM, d1, 1.0), 0.0)
        inv2 = np.where(np.abs(d2) > EPS_DENOM, 1.0 / np.where(np.abs(d2) > EPS_DENOM, d2, 1.0), 0.0)
        B_shift = np.pad(B[:, 1:], ((0, 0), (0, 1)))
        B = (xk - knots[:N_BASIS]) * inv1 * B + (knots[p + 1:p + 1 + N_BASIS] - xk) * inv2 * B_shift
    return B


def _plan(coeffs, knots):
    """Build the evaluation plan: base cubic, paired/single cubic arms,
    extrapolation arms."""
    coeffs = np.asarray(coeffs, np.float64)
    knots = np.asarray(knots, np.float64)
    h = 1.0 / NSEG

    us = np.array([0.125, 0.375, 0.625, 0.875])
    Vinv = np.linalg.inv(np.vander(us, 4, increasing=True))
    g = np.zeros((NSEG, 4))
    for s in range(NSEG):
        xs = (s + us) * h
        g[s] = Vinv @ (_bspline_basis(xs, knots) @ coeffs)
    e = np.zeros(NSEG)
    e[1:] = g[1:, 3] - g[:-1, 3]
    C = [float(c) for c in g[6]]

    # cubic arms in v = w - 6: ("up"/"dn", v_t, kappa, sigma)
    arms = []
    for t in range(7, 13):
        if e[t] != 0.0:
            arms.append(("up", float(t - 6), float(np.cbrt(abs(e[t]))), 1.0 if e[t] > 0 else -1.0))
    for t in range(1, 7):
        if e[t] != 0.0:
            arms.append(("dn", float(t - 6), float(np.cbrt(abs(e[t]))), 1.0 if e[t] > 0 else -1.0))

    ups = sorted([a for a in arms if a[0] == "up"], key=lambda a: a[1])
    dns = sorted([a for a in arms if a[0] == "dn"], key=lambda a: -a[1])
    pairs, singles = [], []
    used = [False] * len(dns)
    for u in ups:
        for i, d in enumerate(dns):
            if not used[i] and d[3] == u[3]:
                used[i] = True
                pairs.append((u, d))
                break
        else:
            singles.append(u)
    singles += [d for i, d in enumerate(dns) if not used[i]]

    def ev(t):
        return float((_bspline_basis(np.array([t]), knots) @ coeffs)[0])

    slope_lo = (ev(0.001) - ev(0.0)) / (0.001 + EPS_DENOM)
    slope_hi = (ev(1.0) - ev(0.999)) / (0.001 + EPS_DENOM)
    return C, pairs, singles, slope_lo, slope_hi


# ---------------------------------------------------------------- device kernel

def _build_nc(plan, nrep=1, cfg=None):
    import concourse.bacc as bacc
    import concourse.mybir as mybir
    from concourse import tile

    cfg = cfg or {}
    n_gp_groups = cfg.get("n_gp_groups", 0)  # groups fully on gpsimd (sq+cube+acc)
    n_sq_gp = cfg.get("n_sq_gp", 0)          # further groups: Square on gpsimd (r*r)
    n_sq_dve = cfg.get("n_sq_dve", 0)        # further groups: Square on DVE (r*r)
    w1_dve = cfg.get("w1_dve", False)        # compute relu(13x) on DVE instead of ACT
    F_ = cfg.get("F", F)
    NT_ = PTS // (P * F_)

    dt = mybir.dt.float32
    op = mybir.AluOpType
    act = mybir.ActivationFunctionType

    C, pairs, singles, slope_lo, slope_hi = plan
    f32 = lambda v: float(np.float32(v))

    # --- precompute all ACT parameters (host, fp64 -> fp32) ---
    # cubic arm groups: list of dicts describing the ACT chain per group
    groups = []     # each: {kind: pair|single, sigma, params...}
    for (u, d) in pairs:
        _, a_vt, k1, sig = u
        _, b_vt, k2, _ = d
        alpha = -(k2 / k1)
        m = (a_vt * k1 + b_vt * k2) / (k1 + k2)
        groups.append(dict(kind="pair", sigma=sig, m=f32(m), alpha=f32(alpha),
                           k1=f32(k1), rbias=f32(-k1 * (a_vt - m))))
    for (side, vt, kap, sig) in singles:
        scale = kap if side == "up" else -kap
        rbias = -kap * vt if side == "up" else kap * vt
        groups.append(dict(kind="single", sigma=sig, scale=f32(scale), rbias=f32(rbias)))

    # extrapolation arms (degree 1): coeff_lo = -slope_lo on relu(-x),
    # coeff_hi = slope_hi on relu(x-1)
    c_lo, c_hi = -slope_lo, slope_hi
    ext = []
    if c_lo != 0.0 and c_hi != 0.0 and (c_lo > 0) == (c_hi > 0):
        klo, khi, sig = abs(c_lo), abs(c_hi), 1.0 if c_lo > 0 else -1.0
        m = khi / (khi + klo)          # (a*khi + b*klo)/(khi+klo), a=1, b=0
        ext.append(dict(kind="pair", sigma=sig, m=f32(m), alpha=f32(-klo / khi),
                        k1=f32(khi), rbias=f32(-khi * (1.0 - m))))
    else:
        if c_lo != 0.0:
            ext.append(dict(kind="single", sigma=1.0 if c_lo > 0 else -1.0,
                            scale=f32(-abs(c_lo)), rbias=0.0))
        if c_hi != 0.0:
            ext.append(dict(kind="single", sigma=1.0 if c_hi > 0 else -1.0,
                            scale=f32(abs(c_hi)), rbias=f32(-abs(c_hi))))

    # collect non-trivial bias constants -> consts tile columns
    bias_vals = []
    def bias_col(val):
        val = f32(val)
        if val not in bias_vals:
            bias_vals.append(val)
        return bias_vals.index(val)
    for grp in groups:
        if grp["kind"] == "pair":
            grp["mcol"] = bias_col(-grp["m"])
        grp["rcol"] = bias_col(grp["rbias"])
    for grp in ext:
        if grp["kind"] == "pair":
            grp["mcol"] = bias_col(-grp["m"])
        grp["rcol"] = bias_col(grp["rbias"])

    nc = bacc.Bacc("TRN2", target_bir_lowering=False, debug=False, num_devices=N_CORES)
    x_ext = nc.dram_tensor("x", [PTS], dt, kind="ExternalInput")
    y_ext = nc.dram_tensor("y", [PTS], dt, kind="ExternalOutput")
    xv = x_ext.ap().rearrange("(n p f) -> n p f", p=P, f=F_)
    yv = y_ext.ap().rearrange("(n p f) -> n p f", p=P, f=F_)

    with tile.TileContext(nc) as tc:
        with (
            tc.tile_pool(name="cp", bufs=1) as cpool,
            tc.tile_pool(name="io", bufs=cfg.get("io_bufs", 3)) as iop,
            tc.tile_pool(name="mid", bufs=cfg.get("mid_bufs", 2)) as midp,
            tc.tile_pool(name="tmp", bufs=cfg.get("tmp_bufs", 3)) as tmpp,
            tc.tile_pool(name="tmp2", bufs=cfg.get("tmp2_bufs", 2)) as tmp2p,
        ):
            consts = cpool.tile([P, max(len(bias_vals), 1)], dt)
            for i, b in enumerate(bias_vals):
                nc.gpsimd.memset(consts[:, i:i + 1], b)

            def bias_ap(col):
                return consts[:, col:col + 1]

            for it in [i for _ in range(nrep) for i in range(NT_)]:
                xt = iop.tile([P, F_], dt, tag="x")
                nc.sync.dma_start(xt[:], xv[it])

                # w1 = relu(13 x) on ACT; v = (w1 min 13) - 6 on DVE
                w1 = midp.tile([P, F_], dt, tag="w1")
                if w1_dve:
                    nc.vector.tensor_scalar(w1[:], xt[:], 13.0, 0.0, op.mult, op.max)
                else:
                    nc.scalar.activation(w1[:], xt[:], act.Relu, bias=0.0, scale=13.0)
                v = midp.tile([P, F_], dt, tag="v")
                nc.vector.tensor_scalar(v[:], w1[:], 13.0, 6.0, op.min, op.subtract)

                # base cubic Horner in h (in place):
                # h = (g3 v + g2); h = h*v; h = (h + g1)*v;  (g0 folded below)
                h = midp.tile([P, F_], dt, tag="h")
                nc.vector.tensor_scalar(h[:], v[:], f32(C[3]), f32(C[2]), op.mult, op.add)
                nc.vector.tensor_tensor(h[:], h[:], v[:], op.mult)
                nc.vector.scalar_tensor_tensor(h[:], h[:], f32(C[1]), v[:], op.add, op.mult)

                # cubic arm groups -> cubes
                cubes = []     # (cube_tile, sigma) accumulated on DVE
                ygp = None     # gpsimd partial sum
                for gi, grp in enumerate(groups):
                    if grp["kind"] == "pair":
                        p_t = tmpp.tile([P, F_], dt, tag="p")
                        nc.scalar.activation(p_t[:], v[:], act.Prelu,
                                             bias=bias_ap(grp["mcol"]), scale=1.0,
                                             alpha=grp["alpha"])
                        r_t = tmpp.tile([P, F_], dt, tag="r")
                        nc.scalar.activation(r_t[:], p_t[:], act.Relu,
                                             bias=bias_ap(grp["rcol"]), scale=grp["k1"])
                    else:
                        r_t = tmpp.tile([P, F_], dt, tag="r")
                        nc.scalar.activation(r_t[:], v[:], act.Relu,
                                             bias=bias_ap(grp["rcol"]), scale=grp["scale"])
                    if gi < n_gp_groups:
                        # whole tail of the group on gpsimd (tensor_tensor only)
                        sq_t = tmp2p.tile([P, F_], dt, tag="sq")
                        nc.gpsimd.tensor_tensor(sq_t[:], r_t[:], r_t[:], op.mult)
                        if ygp is None:
                            ygp = midp.tile([P, F_], dt, tag="ygp")
                            gp_sign = grp["sigma"]
                            nc.gpsimd.tensor_tensor(ygp[:], sq_t[:], r_t[:], op.mult)
                        else:
                            cu_t = tmp2p.tile([P, F_], dt, tag="cu")
                            nc.gpsimd.tensor_tensor(cu_t[:], sq_t[:], r_t[:], op.mult)
                            nc.gpsimd.tensor_tensor(
                                ygp[:], ygp[:], cu_t[:],
                                op.add if grp["sigma"] == gp_sign else op.subtract)
                        continue
                    if gi < n_gp_groups + n_sq_gp:
                        sq_t = tmp2p.tile([P, F_], dt, tag="sq")
                        nc.gpsimd.tensor_tensor(sq_t[:], r_t[:], r_t[:], op.mult)
                    elif gi < n_gp_groups + n_sq_gp + n_sq_dve:
                        sq_t = tmp2p.tile([P, F_], dt, tag="sq")
                        nc.vector.tensor_tensor(sq_t[:], r_t[:], r_t[:], op.mult)
                    else:
                        sq_t = tmp2p.tile([P, F_], dt, tag="sq")
                        nc.scalar.activation(sq_t[:], r_t[:], act.Square)
                    cu_t = tmp2p.tile([P, F_], dt, tag="cu")
                    nc.vector.tensor_tensor(cu_t[:], sq_t[:], r_t[:], op.mult)
                    cubes.append((cu_t, grp["sigma"]))

                # y = (h + g0) +/- cube_0, then accumulate the rest
                y = iop.tile([P, F_], dt, tag="y")
                if cubes:
                    cu0, sig0 = cubes[0]
                    nc.vector.scalar_tensor_tensor(
                        y[:], h[:], f32(C[0]), cu0[:], op.add,
                        op.add if sig0 > 0 else op.subtract)
                    for cu_t, sig in cubes[1:]:
                        nc.vector.tensor_tensor(y[:], y[:], cu_t[:],
                                                op.add if sig > 0 else op.subtract)
                else:
                    nc.vector.tensor_scalar(y[:], h[:], f32(C[0]), None, op.add)
                if ygp is not None:
                    nc.vector.tensor_tensor(y[:], y[:], ygp[:],
                                            op.add if gp_sign > 0 else op.subtract)

                # extrapolation arms (degree 1) on x
                for grp in ext:
                    if grp["kind"] == "pair":
                        p_t = tmpp.tile([P, F_], dt, tag="p")
                        nc.scalar.activation(p_t[:], xt[:], act.Prelu,
                                             bias=bias_ap(grp["mcol"]), scale=1.0,
                                             alpha=grp["alpha"])
                        r_t = tmpp.tile([P, F_], dt, tag="r")
                        nc.scalar.activation(r_t[:], p_t[:], act.Relu,
                                             bias=bias_ap(grp["rcol"]), scale=grp["k1"])
                    else:
                        r_t = tmpp.tile([P, F_], dt, tag="r")
                        nc.scalar.activation(r_t[:], xt[:], act.Relu,
                                             bias=bias_ap(grp["rcol"]), scale=grp["scale"])
                    nc.vector.scalar_tensor_tensor(y[:], r_t[:], grp["sigma"], y[:],
                                                   op.mult, op.add)

                nc.sync.dma_start(yv[it], y[:])

    nc.compile()
    return nc


def _run(x, coeffs, knots, nrep=1, cfg=None, **kw):
    from concourse.bass_utils import run_bass_kernel_spmd

    x = np.ascontiguousarray(np.asarray(x, np.float32).reshape(-1))
    assert x.size == TOTAL, x.size
    plan = _plan(coeffs, knots)
    nc = _build_nc(plan, nrep=nrep, cfg=cfg)

    shards = x.reshape(N_CORES, PTS)
    in_maps = [{"x": shards[i]} for i in range(N_CORES)]
    res = run_bass_kernel_spmd(nc, in_maps, core_ids=list(range(N_CORES)), **kw)
    y = np.concatenate([np.asarray(res.results[i]["y"], np.float32).reshape(-1)
                        for i in range(N_CORES)])
    return y.reshape(-1, 1), res


def kernel(x, coeffs, knots):
    return _run(x, coeffs, knots)[0]
